# revision 27
# baseline (speedup 1.0000x reference)
"""Direct Conv2d (NCHW, OIHW, VALID, stride 1) on 8 Trainium2 NeuronCores.

Problem: input [16, 4, 512, 512] f32, filter [8, 4, 3, 3] f32
         -> output [16, 8, 510, 510] f32.

Sharding: data-parallel over batch N: 2 images per core, filter replicated.

Per-core algorithm (all shapes hardcoded), bf16 end-to-end:
  The tolerance gate (2e-2 relative) admits bf16 data movement (~0.4%
  worst-case error), halving HBM traffic vs fp32.

  STRAIGHT 16-ROW BLOCKS (not the earlier parity-interleaved 30-row
  supertiles): one block computes 16 consecutive output rows for all 8
  output channels with THREE accumulating bf16 matmuls (one per filter
  column shift s, a pure free-dim offset into the shared input tile):

    psum[(j,m), w] += sum_{q,c} lhsT[s][(q,c), (j,m)] * in[c, h0+q, w+s]

  with lhsT[s][(q,c),(j,m)] = filter[m, c, q-j, s] for 0 <= q-j < 3
  (banded matrices, built host-side from the 288-element filter).
  K = 18 input rows x 4 channels = 72 partitions, M = 16 rows x 8
  channels = 128 (FULL output partition dim -- the parity scheme only
  managed 120), N = 510 output columns.  Per output row this is 3/16
  passes vs the parity scheme's 6/30: 192 passes per core instead of
  204, a 2.6 us PE-time saving at the fixed 215 ns/pass cadence.

  Row coverage: blocks b = 0..30 start at h0 = 16b (rows 0..496), the
  tail block starts at h0 = 494 (rows 494..510); rows 494/495 are
  computed twice, bit-identically, landing at DISTINCT device addresses
  (the host unpermute writes them twice).

  PAIR-GRANULAR DMA: blocks are processed in pairs (2p, 2p+1).  The
  host pre-tiles the input as [img, pair, q, c, blk, w] (the 2-row
  inter-block overlap is duplicated host-side, free), so ONE gpsimd
  dma_start loads a [72, 2*512] tile covering both blocks -- one
  contiguous 147 KB DRAM region with 2 KB partition runs.  This halves
  the per-block DMA issue cost on gpsimd (~0.65 us per dma_start,
  regardless of size) to 0.65 us per 1.29 us pair: without pairing the
  issue rate (~0.68 us/load vs 0.645 us/block pace) starves the PE,
  and a single PE idle gap >~1 us mid-stream can demote the HAM clock
  gate to 4/8 -- measured to STICK at 1.2 GHz for the rest of the
  kernel, doubling total time.  Pair 0 is instead loaded as two
  single-block DMAs so the very first matmul is not gated on a
  double-size cold transfer.

  Output: per pair one [128, 1020] SBUF tile (partition (j,m), free
  (blk, w)) stores as one contiguous 261 KB DRAM block [img, pair, j,
  m, blk, w] with 2040-byte partition runs; the host permutes row
  H0[2p+blk]+j back to NCHW.  2 KB runs matter: the HWDGE ring
  generates descriptors at ~17 ns each, so descriptor count sets
  effective per-ring store bandwidth.

  Engine split per pair (PE pace 6 x 215 = 1.29 us):
    - input loads   -> gpsimd (SWDGE Q); nothing else runs there
    - stores        -> sync (SP HWDGE ring)
    - PSUM->SBUF copies (fp32->bf16 cast): DVE for even blocks, ACT for
      odd blocks (DVE-then-ACT writes to the same yt tile don't
      serialize; the reverse order does -- measured)
    - weight load   -> sync, first in program order
  Final pair is stored as two partition-halves on the sync AND gpsimd
  rings in parallel so the closing drain only waits ~1.1 us of
  descriptor generation instead of 2.2.

  HAM warmup: the PE clock-gate defaults to 4/8 (1.2 GHz) and opens to
  8/8 only after ~2.7-3.4 us of sustained matmul activity (free-running
  4096-cycle windows).  A 12-deep accumulation CHAIN of dummy matmuls
  over a zeroed scratch tile spans the weight/input DMA ramp so real
  matmuls start at or near full clock.  The chain must end ~coincident
  with the first input tile's arrival: a PE idle gap of even ~0.8 us
  between warmup and the real stream resets the HAM detector and the
  kernel then runs at 4/8 for several microseconds (or forever).
"""

import os

os.environ.setdefault("MYCRO_LOCAL_CACHE", "1")

import ml_dtypes
import numpy as np

import concourse.bacc as bacc
import concourse.mybir as mybir
import concourse.tile as tile
from concourse.bass_utils import run_bass_kernel_spmd

N_CORES = 8
IMG_PER_CORE = 2
C_IN, H, W = 4, 512, 512
C_OUT, R, S = 8, 3, 3
HO, WO = 510, 510

JB = 16                # output rows per block
QB = JB + 2            # 18 input rows per block
KDIM = C_IN * QB       # 72 matmul contraction partitions
MDIM = C_OUT * JB      # 128 output partitions (full)
NBLK = 32              # blocks per image (31 stride-16 + 1 tail)
NPAIR = NBLK // 2      # load pairs per image
NQUAD = NBLK // 4      # store quads per image
# block row starts: 0, 16, ..., 480, then tail at 494 (rows 494..510;
# rows 494/495 recomputed bit-identically)
H0 = [16 * b for b in range(NBLK - 1)] + [HO - JB]

DT = mybir.dt.bfloat16
NP_DT = ml_dtypes.bfloat16

NWARM = 14   # HAM warmup chain length (see module docstring)

# Set by test harness: TRACE=True -> capture NTFF profile, LAST_EXEC_NS set.
TRACE = False
TRACE_DIR = None
LAST_EXEC_NS = None
LAST_RESULTS = None

_NC_CACHE = {}


def build_wT(filt: np.ndarray) -> np.ndarray:
    """Banded weight matrix [72, S*128] from filter [8, 4, 3, 3].

    K order is q-major (row = q*C_IN + c, q in [0,18)) and M order is
    j-major (col = j*C_OUT + m, j in [0,16)).  Chunk s lives at columns
    [s*128, (s+1)*128); all 128 columns are live (16 rows x 8 ch).
    """
    wT = np.zeros((S, KDIM, MDIM), np.float32)
    for s in range(S):
        for c in range(C_IN):
            for q in range(QB):
                for m in range(C_OUT):
                    for j in range(JB):
                        r = q - j
                        if 0 <= r < R:
                            wT[s, q * C_IN + c, j * C_OUT + m] = filt[m, c, r, s]
    # [partition, (s, col)] so the whole weight set is one contiguous
    # [72, 384] DMA.
    full = wT.transpose(1, 0, 2).reshape(KDIM, S * MDIM)
    return np.ascontiguousarray(full)


def build_x(_input: np.ndarray) -> np.ndarray:
    """Quad-tiled bf16 input [16, NQUAD, QB, C, 4, W] (host side, free).

    x[n, g, q, c, blk, w] = input[n, c, H0[4g+blk] + q, w].  The 2-row
    overlap between neighbouring blocks is duplicated here so every
    device load is one contiguous partition-major region with 4 KB
    partition runs (one max-size SWDGE descriptor per partition).
    """
    xb = _input.astype(NP_DT)  # [16, C, H, W]
    out = np.empty((16, NQUAD, QB, C_IN, 4, W), NP_DT)
    for g in range(NQUAD):
        for blk in range(4):
            h0 = H0[4 * g + blk]
            # [n, c, q, w] -> [n, q, c, w]
            out[:, g, :, :, blk, :] = xb[:, :, h0 : h0 + QB, :].transpose(0, 2, 1, 3)
    return np.ascontiguousarray(out)


def conv_body(tc, y, x, wt_d):
    nc = tc.nc
    with (
        tc.tile_pool(name="wt", bufs=1) as wt_pool,
        tc.tile_pool(name="wu", bufs=1) as wu_pool,
        tc.tile_pool(name="x0", bufs=2) as x0_pool,
        tc.tile_pool(name="xt", bufs=4) as x_pool,
        tc.tile_pool(name="yt", bufs=5) as y_pool,
        tc.tile_pool(name="ps", bufs=7, space="PSUM") as ps_pool,
        tc.tile_pool(name="pw", bufs=1, space="PSUM") as pw_pool,
    ):
        # HAM warmup (see module docstring).  Memset on the otherwise-idle
        # gpsimd engine so the vector engine isn't delayed.  Single
        # ACCUMULATION CHAIN: independent same-bank matmul groups get
        # serialized by Tile with ~0.6 us WAW sem round-trips, but an
        # accumulation chain streams back-to-back.
        wu = wu_pool.tile([128, 320], DT)
        nc.gpsimd.memset(wu[:, :], 0.0)
        pw = pw_pool.tile([128, 320], mybir.dt.float32)
        for k in range(NWARM):
            nc.tensor.matmul(
                pw[:, :],
                lhsT=wu[:, 0:128],
                rhs=wu[:, :],
                start=(k == 0),
                stop=(k == NWARM - 1),
            )
        # Weights: [72, 384]: chunk s at cols [s*128, (s+1)*128).  One DMA
        # on the sync/SP HWDGE ring (idle this early; ACT is busy with
        # framework table loads at startup).
        wt = wt_pool.tile([KDIM, S * MDIM], DT)
        nc.sync.dma_start(out=wt[:, :], in_=wt_d[:, :])
        for i in range(IMG_PER_CORE):
            for g in range(NQUAD):
                lastg = i == IMG_PER_CORE - 1 and g == NQUAD - 1
                # Quad tile: 4 blocks of output, 4080 B partition runs ->
                # single ~4 KB store descriptors (the HWDGE ring generates
                # descriptors at ~10 ns each: 2040 B runs put the ring at
                # 100% duty and the store backlog eventually stalls the
                # PE; 4080 B runs halve it to 50%).
                yt = y_pool.tile([MDIM, 4 * WO], DT)
                # One quad-granular input load: a contiguous 295 KB DRAM
                # block -> partition-major [72, 2048] tile covering all
                # FOUR blocks (partition q*4+c, free blk*512+w, 4 KB
                # partition runs = one max-size descriptor each).  The
                # SWDGE descriptor loop runs at ~20 ns/descriptor: with
                # 2 KB runs (144 desc/quad) the input path is over
                # capacity and starves the PE (HAM then demotes the
                # clock); 4 KB runs (72 desc/quad) put it at ~56%.
                if i == 0 and g == 0:
                    # Quad 0: two pair-half loads so the first matmul is
                    # gated on a 147 KB cold transfer, not 295 KB, and
                    # the quad-1 load issues ~1.4 us earlier.  (2 KB
                    # partition runs here, SWDGE-aggregated 2:1 -- single
                    # 1 KB-run block loads measured 4x the descriptor
                    # count and clogged the queue for 5+ us.)
                    xq0 = x0_pool.tile([KDIM, 2 * W], DT)
                    xq1 = x0_pool.tile([KDIM, 2 * W], DT)
                    nc.gpsimd.dma_start(out=xq0[:, :], in_=x[i, 0, :, :, 0:2, :])
                    nc.gpsimd.dma_start(out=xq1[:, :], in_=x[i, 0, :, :, 2:4, :])
                    rhs4 = [xq0, xq0, xq1, xq1]
                    roff4 = [0, W, 0, W]
                else:
                    xe = x_pool.tile([KDIM, 4 * W], DT)
                    if i == 0 and g == 1:
                        # Quad 1 rides the still-idle sync ring (right
                        # behind the weights): the cold SWDGE pipe can't
                        # move quads 0 AND 1 (590 KB at ~100 GB/s) before
                        # the PE needs them -- the resulting one-time
                        # stall demotes HAM, which never re-promotes
                        # mid-kernel.  This also buys gpsimd a full quad
                        # of head start for quads 2+.
                        nc.sync.dma_start(out=xe[:, :], in_=x[i, g, :, :, :, :])
                    else:
                        nc.gpsimd.dma_start(out=xe[:, :], in_=x[i, g, :, :, :, :])
                    rhs4 = [xe, xe, xe, xe]
                    roff4 = [0, W, 2 * W, 3 * W]
                for s2 in range(4):
                    ps = ps_pool.tile([MDIM, WO], mybir.dt.float32)
                    for s in range(S):
                        o = roff4[s2] + s
                        nc.tensor.matmul(
                            ps[:, :],
                            lhsT=wt[:, s * MDIM : (s + 1) * MDIM],
                            rhs=rhs4[s2][:, o : o + WO],
                            start=(s == 0),
                            stop=(s == S - 1),
                        )
                    # fp32 -> bf16 cast into the quad tile: DVE for
                    # blocks 0-1, ACT for blocks 2-3.  (ACT-after-DVE
                    # writes to one tile run concurrently; the reverse
                    # order serializes -- measured.)
                    if s2 < 2:
                        nc.vector.tensor_copy(
                            yt[:, s2 * WO : (s2 + 1) * WO], ps[:, :]
                        )
                    else:
                        nc.scalar.copy(
                            yt[:, s2 * WO : (s2 + 1) * WO], ps[:, :]
                        )
                # One contiguous 522 KB DRAM block: y[i, g, j, m, blk, w]
                # <-> src partition j*8+m, free blk*510+w (4080 B
                # partition runs).  All steady-state stores ride the
                # sync/SP ring: issuing stores from ACT delays its
                # copies, from gpsimd it delays loads.
                if not lastg:
                    nc.sync.dma_start(
                        out=y[i, g, :, :, :, :],
                        in_=yt[:, :],
                    )
                else:
                    # Final quad: split by partition halves across the
                    # sync AND gpsimd rings (gpsimd has no more loads to
                    # issue) so the closing drain waits ~0.7 us of
                    # descriptor generation instead of 1.3.
                    nc.sync.dma_start(
                        out=y[i, g, 0 : JB // 2, :, :, :],
                        in_=yt[0 : MDIM // 2, :],
                    )
                    nc.gpsimd.dma_start(
                        out=y[i, g, JB // 2 : JB, :, :, :],
                        in_=yt[MDIM // 2 : MDIM, :],
                    )


def build_nc(enable_asserts: bool = False):
    nc = bacc.Bacc(
        "TRN2",
        target_bir_lowering=False,
        debug=False,
        enable_asserts=enable_asserts,
        num_devices=N_CORES,
    )
    # Host quad-tiled input layout (see build_x).
    x = nc.dram_tensor(
        "x", [IMG_PER_CORE, NQUAD, QB, C_IN, 4, W], DT, kind="ExternalInput"
    ).ap()
    wt_d = nc.dram_tensor("wt", [KDIM, S * MDIM], DT, kind="ExternalInput").ap()
    # Device-friendly output layout [img, quad, j, m, blk, w]; host permutes
    # back (row = H0[4g+blk] + j).
    y = nc.dram_tensor(
        "y", [IMG_PER_CORE, NQUAD, JB, C_OUT, 4, WO], DT, kind="ExternalOutput"
    ).ap()
    with tile.TileContext(nc) as tc:
        conv_body(tc, y, x, wt_d)
    nc.compile()
    return nc


def kernel(_input: np.ndarray, _filter: np.ndarray) -> np.ndarray:
    global LAST_EXEC_NS, LAST_RESULTS
    _input = np.asarray(_input, dtype=np.float32)
    _filter = np.asarray(_filter, dtype=np.float32)

    key = DT
    if key not in _NC_CACHE:
        _NC_CACHE[key] = build_nc()
    nc = _NC_CACHE[key]

    wT = build_wT(_filter).astype(NP_DT)
    x_pt = build_x(_input)
    in_maps = [
        {
            "x": x_pt[IMG_PER_CORE * i : IMG_PER_CORE * (i + 1)],
            "wt": wT,
        }
        for i in range(N_CORES)
    ]
    res = run_bass_kernel_spmd(
        nc, in_maps, list(range(N_CORES)), trace=TRACE, tmpdir=TRACE_DIR
    )
    LAST_EXEC_NS = res.exec_time_ns
    LAST_RESULTS = res
    # [img, quad, j, m, blk, w] -> NCHW rows H0[4g+blk]+j, then upcast (host)
    yd = np.concatenate([r["y"] for r in res.results], axis=0).astype(np.float32)
    out = np.empty((16, C_OUT, HO, WO), np.float32)
    for g in range(NQUAD):
        for blk in range(4):
            h0 = H0[4 * g + blk]
            # [n, j, m, w] -> [n, m, j, w]
            out[:, :, h0 : h0 + JB, :] = yd[:, g, :, :, blk, :].transpose(0, 2, 1, 3)
    return out


# revision 29
# speedup vs baseline: 1.6301x; 1.6301x over previous
"""Direct Conv2d (NCHW, OIHW, VALID, stride 1) on 8 Trainium2 NeuronCores.

Problem: input [16, 4, 512, 512] f32, filter [8, 4, 3, 3] f32
         -> output [16, 8, 510, 510] f32.

Sharding: data-parallel over batch N: 2 images per core, filter replicated.

Per-core algorithm (all shapes hardcoded), bf16 end-to-end:
  The tolerance gate (2e-2 relative) admits bf16 data movement (~0.4%
  worst-case error), halving HBM traffic vs fp32.

  STRAIGHT 16-ROW BLOCKS (not the earlier parity-interleaved 30-row
  supertiles): one block computes 16 consecutive output rows for all 8
  output channels with THREE accumulating bf16 matmuls (one per filter
  column shift s, a pure free-dim offset into the shared input tile):

    psum[(j,m), w] += sum_{q,c} lhsT[s][(q,c), (j,m)] * in[c, h0+q, w+s]

  with lhsT[s][(q,c),(j,m)] = filter[m, c, q-j, s] for 0 <= q-j < 3
  (banded matrices, built host-side from the 288-element filter).
  K = 18 input rows x 4 channels = 72 partitions, M = 16 rows x 8
  channels = 128 (FULL output partition dim -- the parity scheme only
  managed 120), N = 510 output columns.  Per output row this is 3/16
  passes vs the parity scheme's 6/30: 192 passes per core instead of
  204, a 2.6 us PE-time saving at the fixed 215 ns/pass cadence.

  Row coverage: blocks b = 0..30 start at h0 = 16b (rows 0..496), the
  tail block starts at h0 = 494 (rows 494..510); rows 494/495 are
  computed twice, bit-identically, landing at DISTINCT device addresses
  (the host unpermute writes them twice).

  PAIR-GRANULAR DMA: blocks are processed in pairs (2p, 2p+1).  The
  host pre-tiles the input as [img, pair, q, c, blk, w] (the 2-row
  inter-block overlap is duplicated host-side, free), so ONE gpsimd
  dma_start loads a [72, 2*512] tile covering both blocks -- one
  contiguous 147 KB DRAM region with 2 KB partition runs.  This halves
  the per-block DMA issue cost on gpsimd (~0.65 us per dma_start,
  regardless of size) to 0.65 us per 1.29 us pair: without pairing the
  issue rate (~0.68 us/load vs 0.645 us/block pace) starves the PE,
  and a single PE idle gap >~1 us mid-stream can demote the HAM clock
  gate to 4/8 -- measured to STICK at 1.2 GHz for the rest of the
  kernel, doubling total time.  Pair 0 is instead loaded as two
  single-block DMAs so the very first matmul is not gated on a
  double-size cold transfer.

  Output: per pair one [128, 1020] SBUF tile (partition (j,m), free
  (blk, w)) stores as one contiguous 261 KB DRAM block [img, pair, j,
  m, blk, w] with 2040-byte partition runs; the host permutes row
  H0[2p+blk]+j back to NCHW.  2 KB runs matter: the HWDGE ring
  generates descriptors at ~17 ns each, so descriptor count sets
  effective per-ring store bandwidth.

  Engine split per pair (PE pace 6 x 215 = 1.29 us):
    - input loads   -> gpsimd (SWDGE Q); nothing else runs there
    - stores        -> sync (SP HWDGE ring)
    - PSUM->SBUF copies (fp32->bf16 cast): DVE for even blocks, ACT for
      odd blocks (DVE-then-ACT writes to the same yt tile don't
      serialize; the reverse order does -- measured)
    - weight load   -> sync, first in program order
  Final pair is stored as two partition-halves on the sync AND gpsimd
  rings in parallel so the closing drain only waits ~1.1 us of
  descriptor generation instead of 2.2.

  HAM warmup: the PE clock-gate defaults to 4/8 (1.2 GHz) and opens to
  8/8 only after ~2.7-3.4 us of sustained matmul activity (free-running
  4096-cycle windows).  A 12-deep accumulation CHAIN of dummy matmuls
  over a zeroed scratch tile spans the weight/input DMA ramp so real
  matmuls start at or near full clock.  The chain must end ~coincident
  with the first input tile's arrival: a PE idle gap of even ~0.8 us
  between warmup and the real stream resets the HAM detector and the
  kernel then runs at 4/8 for several microseconds (or forever).
"""

import os

os.environ.setdefault("MYCRO_LOCAL_CACHE", "1")

import ml_dtypes
import numpy as np

import concourse.bacc as bacc
import concourse.mybir as mybir
import concourse.tile as tile
from concourse.bass_utils import run_bass_kernel_spmd

N_CORES = 8
IMG_PER_CORE = 2
C_IN, H, W = 4, 512, 512
C_OUT, R, S = 8, 3, 3
HO, WO = 510, 510

JB = 16                # output rows per block
QB = JB + 2            # 18 input rows per block
KDIM = C_IN * QB       # 72 matmul contraction partitions
MDIM = C_OUT * JB      # 128 output partitions (full)
NBLK = 32              # blocks per image (31 stride-16 + 1 tail)
NPAIR = NBLK // 2      # load pairs per image
NQUAD = NBLK // 4      # store quads per image
# block row starts: 0, 16, ..., 480, then tail at 494 (rows 494..510;
# rows 494/495 recomputed bit-identically)
H0 = [16 * b for b in range(NBLK - 1)] + [HO - JB]

DT = mybir.dt.bfloat16
NP_DT = ml_dtypes.bfloat16

NWARM = 15   # HAM warmup chain length (see module docstring)

# Set by test harness: TRACE=True -> capture NTFF profile, LAST_EXEC_NS set.
TRACE = False
TRACE_DIR = None
LAST_EXEC_NS = None
LAST_RESULTS = None

_NC_CACHE = {}


def build_wT(filt: np.ndarray) -> np.ndarray:
    """Banded weight matrix [72, S*128] from filter [8, 4, 3, 3].

    K order is q-major (row = q*C_IN + c, q in [0,18)) and M order is
    j-major (col = j*C_OUT + m, j in [0,16)).  Chunk s lives at columns
    [s*128, (s+1)*128); all 128 columns are live (16 rows x 8 ch).
    """
    wT = np.zeros((S, KDIM, MDIM), np.float32)
    for s in range(S):
        for c in range(C_IN):
            for q in range(QB):
                for m in range(C_OUT):
                    for j in range(JB):
                        r = q - j
                        if 0 <= r < R:
                            wT[s, q * C_IN + c, j * C_OUT + m] = filt[m, c, r, s]
    # [partition, (s, col)] so the whole weight set is one contiguous
    # [72, 384] DMA.
    full = wT.transpose(1, 0, 2).reshape(KDIM, S * MDIM)
    return np.ascontiguousarray(full)


def build_x(_input: np.ndarray) -> np.ndarray:
    """Quad-tiled bf16 input [16, NQUAD, QB, C, 4, W] (host side, free).

    x[n, g, q, c, blk, w] = input[n, c, H0[4g+blk] + q, w].  The 2-row
    overlap between neighbouring blocks is duplicated here so every
    device load is one contiguous partition-major region with 4 KB
    partition runs (one max-size SWDGE descriptor per partition).
    """
    xb = _input.astype(NP_DT)  # [16, C, H, W]
    out = np.empty((16, NQUAD, QB, C_IN, 4, W), NP_DT)
    for g in range(NQUAD):
        for blk in range(4):
            h0 = H0[4 * g + blk]
            # [n, c, q, w] -> [n, q, c, w]
            out[:, g, :, :, blk, :] = xb[:, :, h0 : h0 + QB, :].transpose(0, 2, 1, 3)
    return np.ascontiguousarray(out)


def conv_body(tc, y, x, wt_d):
    nc = tc.nc
    with (
        tc.tile_pool(name="wt", bufs=1) as wt_pool,
        tc.tile_pool(name="wu", bufs=1) as wu_pool,
        tc.tile_pool(name="x0", bufs=2) as x0_pool,
        tc.tile_pool(name="xt", bufs=4) as x_pool,
        tc.tile_pool(name="yt", bufs=5) as y_pool,
        tc.tile_pool(name="ps", bufs=7, space="PSUM") as ps_pool,
        tc.tile_pool(name="pw", bufs=1, space="PSUM") as pw_pool,
    ):
        # HAM warmup (see module docstring).  Memset on the otherwise-idle
        # gpsimd engine so the vector engine isn't delayed.  Single
        # ACCUMULATION CHAIN: independent same-bank matmul groups get
        # serialized by Tile with ~0.6 us WAW sem round-trips, but an
        # accumulation chain streams back-to-back.
        wu = wu_pool.tile([128, 320], DT)
        nc.gpsimd.memset(wu[:, :], 0.0)
        pw = pw_pool.tile([128, 320], mybir.dt.float32)
        for k in range(NWARM):
            nc.tensor.matmul(
                pw[:, :],
                lhsT=wu[:, 0:128],
                rhs=wu[:, :],
                start=(k == 0),
                stop=(k == NWARM - 1),
            )
        # Weights: [72, 384]: chunk s at cols [s*128, (s+1)*128).  One DMA
        # on the sync/SP HWDGE ring (idle this early; ACT is busy with
        # framework table loads at startup).
        wt = wt_pool.tile([KDIM, S * MDIM], DT)
        nc.sync.dma_start(out=wt[:, :], in_=wt_d[:, :])
        for i in range(IMG_PER_CORE):
            for g in range(NQUAD):
                lastg = i == IMG_PER_CORE - 1 and g == NQUAD - 1
                # Quad tile: 4 blocks of output, 4080 B partition runs ->
                # single ~4 KB store descriptors (the HWDGE ring generates
                # descriptors at ~10 ns each: 2040 B runs put the ring at
                # 100% duty and the store backlog eventually stalls the
                # PE; 4080 B runs halve it to 50%).
                yt = y_pool.tile([MDIM, 4 * WO], DT)
                # One quad-granular input load: a contiguous 295 KB DRAM
                # block -> partition-major [72, 2048] tile covering all
                # FOUR blocks (partition q*4+c, free blk*512+w, 4 KB
                # partition runs = one max-size descriptor each).  The
                # SWDGE descriptor loop runs at ~20 ns/descriptor: with
                # 2 KB runs (144 desc/quad) the input path is over
                # capacity and starves the PE (HAM then demotes the
                # clock); 4 KB runs (72 desc/quad) put it at ~56%.
                if i == 0 and g == 0:
                    # STARTUP TRANSIENT: the cold DMA paths deliver only
                    # ~50-100 GB/s for the first several us, but the warm
                    # PE wants 114 GB/s of input -- a single path cannot
                    # front-load quads 0-2 (885 KB) in time, and the
                    # resulting one-time PE stall demotes the HAM clock
                    # gate, which NEVER re-promotes mid-kernel (measured:
                    # the whole rest of the kernel then runs at 1.2 GHz).
                    # So the startup spreads across all three DMA paths:
                    #   gpsimd: blocks 0-1, then quad 1, then quads 3+
                    #   sync:   weights, then blocks 2-3
                    #   scalar: quad 2 (one-time; its first copy is much
                    #           later, so no FIFO inversion)
                    # (Blocks 0-1/2-3 as 147 KB pair-halves: 2 KB runs,
                    # SWDGE-aggregated; 1 KB-run single-block loads
                    # measured 4x the descriptors and clog the queue.)
                    xq0 = x0_pool.tile([KDIM, 2 * W], DT)
                    xq1 = x0_pool.tile([KDIM, 2 * W], DT)
                    nc.gpsimd.dma_start(out=xq0[:, :], in_=x[i, 0, :, :, 0:2, :])
                    nc.sync.dma_start(out=xq1[:, :], in_=x[i, 0, :, :, 2:4, :])
                    rhs4 = [xq0, xq0, xq1, xq1]
                    roff4 = [0, W, 0, W]
                else:
                    xe = x_pool.tile([KDIM, 4 * W], DT)
                    if i == 0 and g == 2:
                        nc.scalar.dma_start(out=xe[:, :], in_=x[i, g, :, :, :, :])
                    else:
                        nc.gpsimd.dma_start(out=xe[:, :], in_=x[i, g, :, :, :, :])
                    rhs4 = [xe, xe, xe, xe]
                    roff4 = [0, W, 2 * W, 3 * W]
                for s2 in range(4):
                    ps = ps_pool.tile([MDIM, WO], mybir.dt.float32)
                    for s in range(S):
                        o = roff4[s2] + s
                        nc.tensor.matmul(
                            ps[:, :],
                            lhsT=wt[:, s * MDIM : (s + 1) * MDIM],
                            rhs=rhs4[s2][:, o : o + WO],
                            start=(s == 0),
                            stop=(s == S - 1),
                        )
                    # fp32 -> bf16 cast into the quad tile: DVE for
                    # blocks 0-1, ACT for blocks 2-3.  (ACT-after-DVE
                    # writes to one tile run concurrently; the reverse
                    # order serializes -- measured.)
                    if s2 < 2:
                        nc.vector.tensor_copy(
                            yt[:, s2 * WO : (s2 + 1) * WO], ps[:, :]
                        )
                    else:
                        nc.scalar.copy(
                            yt[:, s2 * WO : (s2 + 1) * WO], ps[:, :]
                        )
                # One contiguous 522 KB DRAM block: y[i, g, j, m, blk, w]
                # <-> src partition j*8+m, free blk*510+w (4080 B
                # partition runs).  All steady-state stores ride the
                # sync/SP ring: issuing stores from ACT delays its
                # copies, from gpsimd it delays loads.
                if not lastg:
                    nc.sync.dma_start(
                        out=y[i, g, :, :, :, :],
                        in_=yt[:, :],
                    )
                else:
                    # Final quad: split by partition halves across the
                    # sync AND gpsimd rings (gpsimd has no more loads to
                    # issue) so the closing drain waits ~0.7 us of
                    # descriptor generation instead of 1.3.
                    nc.sync.dma_start(
                        out=y[i, g, 0 : JB // 2, :, :, :],
                        in_=yt[0 : MDIM // 2, :],
                    )
                    nc.gpsimd.dma_start(
                        out=y[i, g, JB // 2 : JB, :, :, :],
                        in_=yt[MDIM // 2 : MDIM, :],
                    )


def build_nc(enable_asserts: bool = False):
    nc = bacc.Bacc(
        "TRN2",
        target_bir_lowering=False,
        debug=False,
        enable_asserts=enable_asserts,
        num_devices=N_CORES,
    )
    # Host quad-tiled input layout (see build_x).
    x = nc.dram_tensor(
        "x", [IMG_PER_CORE, NQUAD, QB, C_IN, 4, W], DT, kind="ExternalInput"
    ).ap()
    wt_d = nc.dram_tensor("wt", [KDIM, S * MDIM], DT, kind="ExternalInput").ap()
    # Device-friendly output layout [img, quad, j, m, blk, w]; host permutes
    # back (row = H0[4g+blk] + j).
    y = nc.dram_tensor(
        "y", [IMG_PER_CORE, NQUAD, JB, C_OUT, 4, WO], DT, kind="ExternalOutput"
    ).ap()
    with tile.TileContext(nc) as tc:
        conv_body(tc, y, x, wt_d)
    nc.compile()
    return nc


def kernel(_input: np.ndarray, _filter: np.ndarray) -> np.ndarray:
    global LAST_EXEC_NS, LAST_RESULTS
    _input = np.asarray(_input, dtype=np.float32)
    _filter = np.asarray(_filter, dtype=np.float32)

    key = DT
    if key not in _NC_CACHE:
        _NC_CACHE[key] = build_nc()
    nc = _NC_CACHE[key]

    wT = build_wT(_filter).astype(NP_DT)
    x_pt = build_x(_input)
    in_maps = [
        {
            "x": x_pt[IMG_PER_CORE * i : IMG_PER_CORE * (i + 1)],
            "wt": wT,
        }
        for i in range(N_CORES)
    ]
    res = run_bass_kernel_spmd(
        nc, in_maps, list(range(N_CORES)), trace=TRACE, tmpdir=TRACE_DIR
    )
    LAST_EXEC_NS = res.exec_time_ns
    LAST_RESULTS = res
    # [img, quad, j, m, blk, w] -> NCHW rows H0[4g+blk]+j, then upcast (host)
    yd = np.concatenate([r["y"] for r in res.results], axis=0).astype(np.float32)
    out = np.empty((16, C_OUT, HO, WO), np.float32)
    for g in range(NQUAD):
        for blk in range(4):
            h0 = H0[4 * g + blk]
            # [n, j, m, w] -> [n, m, j, w]
            out[:, :, h0 : h0 + JB, :] = yd[:, g, :, :, blk, :].transpose(0, 2, 1, 3)
    return out
